# revision 7
# baseline (speedup 1.0000x reference)
"""e3nn-style 5x5x5 SAME conv3d ([2,32,32,32,32] -> [2,32,32,32,288]) on 8 trn2 cores.

Sharding: batch(2) x X-chunks(4) -> 8 cores; each core gets a zero-padded input
slab with 2-voxel halos and computes its [8,32,32,288] output slice.

Per-core algorithm (implicit GEMM, fp32r):
  - The 125-tap conv contracts over (tap, c_in) = 4000. Taps are packed 4-per-
    matmul into K=128 by replicating the input slab 4x across partition groups
    with shifts: slab A shifts z by r (covers taps dz=0..3), slab B shifts y by
    r at dz=4 (covers the dz=4 plane with 2 y-bar matmuls per dx). 35 matmuls
    of [K=128, M=128 voxels] x [K=128, N=288 out-ch] accumulate per PSUM block.
  - The 5x5x5x32x288 conv kernel is synthesized on device from the tiny radial
    weights: per (l, v, r) a [8 k, 32 u] x [8 k, 35*ml] matmul against host
    constants EY[k, tap-selected, m] = emb[t,k]*Y_l[t,m]/(nvox*fan), written to
    the 4 partition groups via tile_position col-tiling, then copied into the
    conv-weight layout. The center tap is replaced by w_lin/fan (l=0 block).
"""

import numpy as np

try:
    import concourse.bass as bass  # noqa: F401
except ImportError:
    import sys

    sys.path.insert(0, "/opt/trn_rl_repo")

import concourse.mybir as mybir
import concourse.tile as tile
from concourse import bacc
from concourse.bass_utils import run_bass_kernel_spmd

F32 = mybir.dt.float32
F32R = mybir.dt.float32r

GRID = 32
CIN = 32
COUT = 288
NRB = 8
XPER = 8  # output x-planes per core
XS = 12  # slab x extent (XPER + 2*2 halo)
MLS = (1, 3, 5)
LOFF_EY = (0, 36, 144)  # l-block offsets in the 324-wide EY free dim (36 bases)
LOFF_OUT = (0, 32, 128)  # l-block offsets in the 288 output channels
FAN = float(np.sqrt(32.0))
NVOX = 125.0


def _tap_of(b, r):
    """Flat tap index covered by weight-base b at partition group r, or None."""
    if b >= 35:
        return None  # pad base (even-N requirement for fp32r matmul)
    if b < 25:
        dx, dy = divmod(b, 5)
        if (dx, dy, r) == (2, 2, 2):
            return None  # center tap: replaced by w_lin
        return (dx * 5 + dy) * 5 + r  # dz = r
    dx, yb = divmod(b - 25, 2)
    if yb == 1 and r < 3:
        return None  # duplicate of the yb=0 bar
    dy = yb + r
    return (dx * 5 + dy) * 5 + 4  # dz = 4


def _host_consts():
    """EY[k, r, col]: tap-selected emb x spherical-harmonic products (f32)."""
    c = np.arange(-2.0, 3.0)
    lat = np.stack(np.meshgrid(c, c, c, indexing="ij"), axis=-1).reshape(125, 3)
    rad = np.linalg.norm(lat, axis=-1)
    u = lat / np.where(rad == 0.0, 1.0, rad)[:, None]
    ux, uy, uz = u[:, 0], u[:, 1], u[:, 2]

    y0 = np.ones((125, 1))
    y1 = np.sqrt(3.0) * np.stack([uy, uz, ux], axis=-1)
    y2 = np.stack(
        [
            np.sqrt(15.0) * ux * uy,
            np.sqrt(15.0) * uy * uz,
            (np.sqrt(5.0) / 2.0) * (2.0 * uz**2 - ux**2 - uy**2),
            np.sqrt(15.0) * ux * uz,
            (np.sqrt(15.0) / 2.0) * (ux**2 - uy**2),
        ],
        axis=-1,
    )
    ys = (y0, y1, y2)

    # e3nn soft_one_hot_linspace, basis='smooth_finite'
    values = np.linspace(0.0, 2.5, NRB + 2)
    step = values[1] - values[0]
    values = values[1:-1]
    d = (rad[:, None] - values[None, :]) / step

    def sus(x):
        return np.where(x > 0.0, np.exp(-1.0 / np.where(x > 0.0, x, 1.0)), 0.0)

    emb = 1.14136 * np.exp(2.0) * sus(d + 1.0) * sus(1.0 - d)  # [125, 8]
    emb = emb / (NVOX * FAN)

    # EY4[r*8+k, col]: block-diag-matched moving operand, per l-block of 36
    # bases (b=35 is a zero pad column group).
    ey = np.zeros((32, 324), np.float32)
    for r in range(4):
        for l in range(3):
            ml = MLS[l]
            for b in range(36):
                t = _tap_of(b, r)
                if t is None:
                    continue
                col = LOFF_EY[l] + b * ml
                ey[r * 8 : r * 8 + 8, col : col + ml] = (
                    emb[t, :, None] * ys[l][t, None, :]
                ).astype(np.float32)
    return ey


def _build_nc():
    nc = bacc.Bacc("TRN2", target_bir_lowering=False, debug=False)

    pa_d = nc.dram_tensor("pa", [128, XS, 36, 32], F32R, kind="ExternalInput")
    pb_d = nc.dram_tensor("pb", [128, XS, 33, 32], F32R, kind="ExternalInput")
    w_d = [
        nc.dram_tensor(f"w{l}t", [NRB, 32, 32], F32R, kind="ExternalInput")
        for l in range(3)
    ]
    wlin_d = nc.dram_tensor("wlin", [32, 32], F32, kind="ExternalInput")
    ey_d = nc.dram_tensor("ey", [32, 324], F32R, kind="ExternalInput")
    out_d = nc.dram_tensor("out", [XPER * 32 * 32, COUT], F32, kind="ExternalOutput")

    with tile.TileContext(nc) as tc:
        with (
            tc.tile_pool(name="wpool", bufs=1) as wpool,
            tc.tile_pool(name="rall", bufs=1) as rall_pool,
            tc.tile_pool(name="slab", bufs=1) as slab_pool,
            tc.tile_pool(name="stage", bufs=3) as stage_pool,
            tc.tile_pool(name="ps", bufs=6, space="PSUM") as ps_pool,
        ):
            # --- tiny inputs ---
            ey_sb = wpool.tile([32, 324], F32R, tag="ey", name="ey_sb")
            nc.sync.dma_start(ey_sb[:], ey_d[:])
            wlin_sb = wpool.tile([128, 32], F32, tag="wlin", name="wlin_sb")
            nc.sync.dma_start(wlin_sb[64:96, :], wlin_d[:])

            # Block-diagonal stationary: BD_l[8r+k, v*128 + 32r+u] = w_l[k,u,v]
            bd_sb = []
            for l in range(3):
                bd_l = wpool.tile([32, 32 * 128], F32R, tag=f"bd{l}", name=f"bd{l}")
                nc.vector.memset(bd_l[:].bitcast(F32), 0.0)
                bd3 = bd_l.rearrange("p (v q) -> p v q", q=128)
                for r in range(4):
                    nc.sync.dma_start(
                        bd3[8 * r : 8 * r + 8, :, 32 * r : 32 * r + 32], w_d[l][:]
                    )
                bd_sb.append(bd_l)

            # --- conv-weight synthesis into R_all[(r,u), (b, outch)] ---
            # one fp32r matmul per (l, v): [K=32 BD] x [K=32, N=36*ml EY] ->
            # psum[(r,u), (b, m)], then a strided DVE copy into R_all.
            r_all = rall_pool.tile([128, 35 * COUT], F32R, tag="rall", name="r_all")
            r_view = r_all.rearrange("p (b c) -> p b c", c=COUT)
            for l in range(3):
                ml = MLS[l]
                for v in range(32):
                    ps_lv = ps_pool.tile([128, 36 * ml], F32, tag="ps", name="ps_syn")
                    nc.tensor.matmul(
                        ps_lv[:, :],
                        bd_sb[l][:, 128 * v : 128 * (v + 1)],
                        ey_sb[:, LOFF_EY[l] : LOFF_EY[l] + 36 * ml],
                        start=True,
                        stop=True,
                    )
                    nc.vector.tensor_copy(
                        r_view[:, :, LOFF_OUT[l] + v * ml : LOFF_OUT[l] + (v + 1) * ml],
                        ps_lv.rearrange("p (b m) -> p b m", m=ml)[:, :35, :],
                    )
            # center tap (2,2,2) -> base b=12, group r=2, l=0 block
            nc.scalar.mul(
                r_view[64:96, 12, 0:32],
                wlin_sb[64:96, :],
                1.0 / FAN,
            )

            # --- input slabs (4x partition-replicated with shifts) ---
            a_sb = []
            b_sb = []
            for x in range(XS):
                a_x = slab_pool.tile([128, 36, 32], F32R, tag=f"A{x}", name=f"a{x}")
                nc.sync.dma_start(a_x[:], pa_d[:, x, :, :])
                a_sb.append(a_x)
                b_x = slab_pool.tile([128, 33, 32], F32R, tag=f"B{x}", name=f"b{x}")
                nc.sync.dma_start(b_x[:], pb_d[:, x, :, :])
                b_sb.append(b_x)

            # --- conv main loop: 64 blocks x 35 accumulating matmuls ---
            for xo in range(XPER):
                for yb in range(8):
                    ps_blk = ps_pool.tile([128, COUT], F32, tag="ps", name="ps_blk")
                    mm = 0
                    for dx in range(5):
                        for dy in range(5):
                            b = dx * 5 + dy
                            y0 = 4 * yb + dy
                            nc.tensor.matmul(
                                ps_blk[:, :],
                                a_sb[xo + dx][:, y0 : y0 + 4, :],
                                r_all[:, b * COUT : (b + 1) * COUT],
                                start=(mm == 0),
                                stop=False,
                            )
                            mm += 1
                    for dx in range(5):
                        for yb2 in range(2):
                            b = 25 + dx * 2 + yb2
                            y0 = 4 * yb + yb2
                            mm += 1
                            nc.tensor.matmul(
                                ps_blk[:, :],
                                b_sb[xo + dx][:, y0 : y0 + 4, :],
                                r_all[:, b * COUT : (b + 1) * COUT],
                                start=False,
                                stop=(mm == 35),
                            )
                    stg = stage_pool.tile([128, COUT], F32, tag="stg", name="stg")
                    nc.vector.tensor_copy(stg[:], ps_blk[:])
                    row = xo * 1024 + yb * 128
                    nc.sync.dma_start(out_d[row : row + 128, :], stg[:])

    nc.compile()
    return nc


def _shard_inputs(x, w0, w1, w2, w_lin):
    ey = _host_consts()
    wts = [
        np.ascontiguousarray(w.transpose(0, 2, 1)).astype(np.float32)
        for w in (w0, w1, w2)
    ]
    w_lin = np.ascontiguousarray(w_lin).astype(np.float32)
    in_maps = []
    for core in range(8):
        bb, xi = divmod(core, 4)
        x0 = xi * XPER
        pp = np.zeros((CIN, XS, 36, 36), np.float32)
        glo, ghi = x0 - 2, x0 + XPER + 2
        slo, shi = max(glo, 0), min(ghi, GRID)
        pp[:, slo - glo : shi - glo, 2:34, 2:34] = x[bb, slo:shi].transpose(3, 0, 1, 2)
        p4a = np.stack([pp[:, :, :, r : r + 32] for r in range(4)], axis=0)
        p4b = np.stack([pp[:, :, r : r + 33, 4:36] for r in range(4)], axis=0)
        in_maps.append(
            {
                "pa": np.ascontiguousarray(p4a).reshape(128, XS, 36, 32),
                "pb": np.ascontiguousarray(p4b).reshape(128, XS, 33, 32),
                "w0t": wts[0],
                "w1t": wts[1],
                "w2t": wts[2],
                "wlin": w_lin,
                "ey": ey,
            }
        )
    return in_maps


_NC = None


def _run(x, w0, w1, w2, w_lin, **spmd_kwargs):
    global _NC
    if _NC is None:
        _NC = _build_nc()
    in_maps = _shard_inputs(
        np.asarray(x, np.float32),
        np.asarray(w0, np.float32),
        np.asarray(w1, np.float32),
        np.asarray(w2, np.float32),
        np.asarray(w_lin, np.float32),
    )
    res = run_bass_kernel_spmd(_NC, in_maps, core_ids=list(range(8)), **spmd_kwargs)
    out = np.empty((2, GRID, GRID, GRID, COUT), np.float32)
    for core in range(8):
        bb, xi = divmod(core, 4)
        out[bb, xi * XPER : (xi + 1) * XPER] = res.results[core]["out"].reshape(
            XPER, GRID, GRID, COUT
        )
    return out, res


def kernel(x, w0, w1, w2, w_lin):
    out, _ = _run(x, w0, w1, w2, w_lin)
    return out


# revision 9
# speedup vs baseline: 21.2477x; 21.2477x over previous
"""e3nn-style 5x5x5 SAME conv3d ([2,32,32,32,32] -> [2,32,32,32,288]) on 8 trn2 cores.

Sharding: batch(2) x X-chunks(4) -> 8 cores; each core gets a zero-padded input
slab with 2-voxel halos and computes its [8,32,32,288] output slice.

Per-core algorithm (implicit GEMM, fp32r):
  - The 125-tap conv contracts over (tap, c_in) = 4000. Taps are packed 4-per-
    matmul into K=128 by replicating the input slab 4x across partition groups
    with shifts: slab A shifts z by r (covers taps dz=0..3), slab B shifts y by
    r at dz=4 (covers the dz=4 plane with 2 y-bar matmuls per dx). 35 matmuls
    of [K=128, M=128 voxels] x [K=128, N=288 out-ch] accumulate per PSUM block.
  - The 5x5x5x32x288 conv kernel is synthesized on device from the tiny radial
    weights: per (l, v, r) a [8 k, 32 u] x [8 k, 35*ml] matmul against host
    constants EY[k, tap-selected, m] = emb[t,k]*Y_l[t,m]/(nvox*fan), written to
    the 4 partition groups via tile_position col-tiling, then copied into the
    conv-weight layout. The center tap is replaced by w_lin/fan (l=0 block).
"""

import numpy as np

try:
    import concourse.bass as bass  # noqa: F401
except ImportError:
    import sys

    sys.path.insert(0, "/opt/trn_rl_repo")

import concourse.mybir as mybir
import concourse.tile as tile
from concourse import bacc
from concourse.bass_utils import run_bass_kernel_spmd

F32 = mybir.dt.float32
F32R = mybir.dt.float32r

GRID = 32
CIN = 32
COUT = 288
NRB = 8
XPER = 8  # output x-planes per core
XS = 12  # slab x extent (XPER + 2*2 halo)
MLS = (1, 3, 5)
LOFF_EY = (0, 36, 144)  # l-block offsets in the 324-wide EY free dim (36 bases)
LOFF_OUT = (0, 32, 128)  # l-block offsets in the 288 output channels
FAN = float(np.sqrt(32.0))
NVOX = 125.0


def _tap_of(b, r):
    """Flat tap index covered by weight-base b at partition group r, or None."""
    if b >= 35:
        return None  # pad base (even-N requirement for fp32r matmul)
    if b < 25:
        dx, dy = divmod(b, 5)
        if (dx, dy, r) == (2, 2, 2):
            return None  # center tap: replaced by w_lin
        return (dx * 5 + dy) * 5 + r  # dz = r
    dx, yb = divmod(b - 25, 2)
    if yb == 1 and r < 3:
        return None  # duplicate of the yb=0 bar
    dy = yb + r
    return (dx * 5 + dy) * 5 + 4  # dz = 4


def _host_consts():
    """EY[k, r, col]: tap-selected emb x spherical-harmonic products (f32)."""
    c = np.arange(-2.0, 3.0)
    lat = np.stack(np.meshgrid(c, c, c, indexing="ij"), axis=-1).reshape(125, 3)
    rad = np.linalg.norm(lat, axis=-1)
    u = lat / np.where(rad == 0.0, 1.0, rad)[:, None]
    ux, uy, uz = u[:, 0], u[:, 1], u[:, 2]

    y0 = np.ones((125, 1))
    y1 = np.sqrt(3.0) * np.stack([uy, uz, ux], axis=-1)
    y2 = np.stack(
        [
            np.sqrt(15.0) * ux * uy,
            np.sqrt(15.0) * uy * uz,
            (np.sqrt(5.0) / 2.0) * (2.0 * uz**2 - ux**2 - uy**2),
            np.sqrt(15.0) * ux * uz,
            (np.sqrt(15.0) / 2.0) * (ux**2 - uy**2),
        ],
        axis=-1,
    )
    ys = (y0, y1, y2)

    # e3nn soft_one_hot_linspace, basis='smooth_finite'
    values = np.linspace(0.0, 2.5, NRB + 2)
    step = values[1] - values[0]
    values = values[1:-1]
    d = (rad[:, None] - values[None, :]) / step

    def sus(x):
        return np.where(x > 0.0, np.exp(-1.0 / np.where(x > 0.0, x, 1.0)), 0.0)

    emb = 1.14136 * np.exp(2.0) * sus(d + 1.0) * sus(1.0 - d)  # [125, 8]
    emb = emb / (NVOX * FAN)

    # EY4[r*8+k, col]: block-diag-matched moving operand, per l-block of 36
    # bases (b=35 is a zero pad column group).
    ey = np.zeros((32, 324), np.float32)
    for r in range(4):
        for l in range(3):
            ml = MLS[l]
            for b in range(36):
                t = _tap_of(b, r)
                if t is None:
                    continue
                col = LOFF_EY[l] + b * ml
                ey[r * 8 : r * 8 + 8, col : col + ml] = (
                    emb[t, :, None] * ys[l][t, None, :]
                ).astype(np.float32)
    return ey


def _build_nc(repeat=1):
    nc = bacc.Bacc("TRN2", target_bir_lowering=False, debug=False)

    pa_d = nc.dram_tensor("pa", [128, XS, 36, 32], F32R, kind="ExternalInput")
    pb_d = nc.dram_tensor("pb", [128, XS, 33, 32], F32R, kind="ExternalInput")
    w_d = [
        nc.dram_tensor(f"w{l}t", [NRB, 32, 32], F32R, kind="ExternalInput")
        for l in range(3)
    ]
    wlin_d = nc.dram_tensor("wlin", [32, 32], F32, kind="ExternalInput")
    ey_d = nc.dram_tensor("ey", [32, 324], F32R, kind="ExternalInput")
    out_d = nc.dram_tensor("out", [XPER * 32 * 32, COUT], F32, kind="ExternalOutput")

    with tile.TileContext(nc) as tc:
        with (
            tc.tile_pool(name="wpool", bufs=1) as wpool,
            tc.tile_pool(name="rall", bufs=1) as rall_pool,
            tc.tile_pool(name="slab", bufs=1) as slab_pool,
            tc.tile_pool(name="stage", bufs=3) as stage_pool,
            tc.tile_pool(name="ps", bufs=6, space="PSUM") as ps_pool,
        ):
            # --- tiny inputs ---
            ey_sb = wpool.tile([32, 324], F32R, tag="ey", name="ey_sb")
            nc.sync.dma_start(ey_sb[:], ey_d[:])
            wlin_sb = wpool.tile([128, 32], F32, tag="wlin", name="wlin_sb")
            nc.sync.dma_start(wlin_sb[64:96, :], wlin_d[:])

            # Block-diagonal stationary: BD_l[8r+k, v*128 + 32r+u] = w_l[k,u,v]
            bd_sb = []
            for l in range(3):
                bd_l = wpool.tile([32, 32 * 128], F32R, tag=f"bd{l}", name=f"bd{l}")
                nc.vector.memset(bd_l[:].bitcast(F32), 0.0)
                bd3 = bd_l.rearrange("p (v q) -> p v q", q=128)
                for r in range(4):
                    nc.sync.dma_start(
                        bd3[8 * r : 8 * r + 8, :, 32 * r : 32 * r + 32], w_d[l][:]
                    )
                bd_sb.append(bd_l)

            # --- conv-weight synthesis into R_all[(r,u), (b, outch)] ---
            # one fp32r matmul per (l, v): [K=32 BD] x [K=32, N=36*ml EY] ->
            # psum[(r,u), (b, m)], then a strided DVE copy into R_all.
            r_all = rall_pool.tile([128, 35 * COUT], F32R, tag="rall", name="r_all")
            r_view = r_all.rearrange("p (b c) -> p b c", c=COUT)
            for l in range(3):
                ml = MLS[l]
                for v in range(32):
                    ps_lv = ps_pool.tile([128, 36 * ml], F32, tag="ps", name="ps_syn")
                    nc.tensor.matmul(
                        ps_lv[:, :],
                        bd_sb[l][:, 128 * v : 128 * (v + 1)],
                        ey_sb[:, LOFF_EY[l] : LOFF_EY[l] + 36 * ml],
                        start=True,
                        stop=True,
                    )
                    nc.vector.tensor_copy(
                        r_view[:, :, LOFF_OUT[l] + v * ml : LOFF_OUT[l] + (v + 1) * ml],
                        ps_lv.rearrange("p (b m) -> p b m", m=ml)[:, :35, :],
                    )
            # center tap (2,2,2) -> base b=12, group r=2, l=0 block
            nc.scalar.mul(
                r_view[64:96, 12, 0:32],
                wlin_sb[64:96, :],
                1.0 / FAN,
            )

            # --- input slabs (4x partition-replicated with shifts) ---
            a_sb = []
            b_sb = []
            for x in range(XS):
                a_x = slab_pool.tile([128, 36, 32], F32R, tag=f"A{x}", name=f"a{x}")
                nc.sync.dma_start(a_x[:], pa_d[:, x, :, :])
                a_sb.append(a_x)
                b_x = slab_pool.tile([128, 33, 32], F32R, tag=f"B{x}", name=f"b{x}")
                nc.sync.dma_start(b_x[:], pb_d[:, x, :, :])
                b_sb.append(b_x)

            # --- conv main loop: 64 blocks x 35 accumulating matmuls ---
            # (repeat>1 re-runs the whole conv pass; benchmarking only)
            for _rep in range(repeat):
              for xo in range(XPER):
                for yb in range(8):
                    ps_blk = ps_pool.tile([128, COUT], F32, tag="ps", name="ps_blk")
                    mm = 0
                    for dx in range(5):
                        for dy in range(5):
                            b = dx * 5 + dy
                            y0 = 4 * yb + dy
                            nc.tensor.matmul(
                                ps_blk[:, :],
                                a_sb[xo + dx][:, y0 : y0 + 4, :],
                                r_all[:, b * COUT : (b + 1) * COUT],
                                start=(mm == 0),
                                stop=False,
                            )
                            mm += 1
                    for dx in range(5):
                        for yb2 in range(2):
                            b = 25 + dx * 2 + yb2
                            y0 = 4 * yb + yb2
                            mm += 1
                            nc.tensor.matmul(
                                ps_blk[:, :],
                                b_sb[xo + dx][:, y0 : y0 + 4, :],
                                r_all[:, b * COUT : (b + 1) * COUT],
                                start=False,
                                stop=(mm == 35),
                            )
                    stg = stage_pool.tile([128, COUT], F32, tag="stg", name="stg")
                    nc.vector.tensor_copy(stg[:], ps_blk[:])
                    row = xo * 1024 + yb * 128
                    nc.sync.dma_start(out_d[row : row + 128, :], stg[:])

    nc.compile()
    return nc


def _shard_inputs(x, w0, w1, w2, w_lin):
    ey = _host_consts()
    wts = [
        np.ascontiguousarray(w.transpose(0, 2, 1)).astype(np.float32)
        for w in (w0, w1, w2)
    ]
    w_lin = np.ascontiguousarray(w_lin).astype(np.float32)
    in_maps = []
    for core in range(8):
        bb, xi = divmod(core, 4)
        x0 = xi * XPER
        pp = np.zeros((CIN, XS, 36, 36), np.float32)
        glo, ghi = x0 - 2, x0 + XPER + 2
        slo, shi = max(glo, 0), min(ghi, GRID)
        pp[:, slo - glo : shi - glo, 2:34, 2:34] = x[bb, slo:shi].transpose(3, 0, 1, 2)
        p4a = np.stack([pp[:, :, :, r : r + 32] for r in range(4)], axis=0)
        p4b = np.stack([pp[:, :, r : r + 33, 4:36] for r in range(4)], axis=0)
        in_maps.append(
            {
                "pa": np.ascontiguousarray(p4a).reshape(128, XS, 36, 32),
                "pb": np.ascontiguousarray(p4b).reshape(128, XS, 33, 32),
                "w0t": wts[0],
                "w1t": wts[1],
                "w2t": wts[2],
                "wlin": w_lin,
                "ey": ey,
            }
        )
    return in_maps


_NC = None


def _run(x, w0, w1, w2, w_lin, **spmd_kwargs):
    global _NC
    if _NC is None:
        _NC = _build_nc()
    in_maps = _shard_inputs(
        np.asarray(x, np.float32),
        np.asarray(w0, np.float32),
        np.asarray(w1, np.float32),
        np.asarray(w2, np.float32),
        np.asarray(w_lin, np.float32),
    )
    res = run_bass_kernel_spmd(_NC, in_maps, core_ids=list(range(8)), **spmd_kwargs)
    out = np.empty((2, GRID, GRID, GRID, COUT), np.float32)
    for core in range(8):
        bb, xi = divmod(core, 4)
        out[bb, xi * XPER : (xi + 1) * XPER] = res.results[core]["out"].reshape(
            XPER, GRID, GRID, COUT
        )
    return out, res


def kernel(x, w0, w1, w2, w_lin):
    out, _ = _run(x, w0, w1, w2, w_lin)
    return out


# revision 11
# speedup vs baseline: 710.0479x; 33.4176x over previous
"""e3nn-style 5x5x5 SAME conv3d ([2,32,32,32,32] -> [2,32,32,32,288]) on 8 trn2 cores.

Sharding: batch(2) x X-chunks(4) -> 8 cores; each core gets a zero-padded input
slab with 2-voxel halos and computes its [8,32,32,288] output slice.

Per-core algorithm (implicit GEMM, fp32r):
  - The 125-tap conv contracts over (tap, c_in) = 4000. Taps are packed 4-per-
    matmul into K=128 by replicating the input slab 4x across partition groups
    with shifts: slab A shifts z by r (covers taps dz=0..3), slab B shifts y by
    r at dz=4 (covers the dz=4 plane with 2 y-bar matmuls per dx). 35 matmuls
    of [K=128, M=128 voxels] x [K=128, N=288 out-ch] accumulate per PSUM block.
  - The 5x5x5x32x288 conv kernel is synthesized on device from the tiny radial
    weights: per (l, v, r) a [8 k, 32 u] x [8 k, 35*ml] matmul against host
    constants EY[k, tap-selected, m] = emb[t,k]*Y_l[t,m]/(nvox*fan), written to
    the 4 partition groups via tile_position col-tiling, then copied into the
    conv-weight layout. The center tap is replaced by w_lin/fan (l=0 block).
"""

import numpy as np

try:
    import concourse.bass as bass  # noqa: F401
except ImportError:
    import sys

    sys.path.insert(0, "/opt/trn_rl_repo")

import concourse.mybir as mybir
import concourse.tile as tile
from concourse import bacc
from concourse.bass_utils import run_bass_kernel_spmd

F32 = mybir.dt.float32
F32R = mybir.dt.float32r
F16 = mybir.dt.float16

GRID = 32
CIN = 32
COUT = 288
NRB = 8
XPER = 8  # output x-planes per core
XS = 12  # slab x extent (XPER + 2*2 halo)
MLS = (1, 3, 5)
LOFF_EY = (0, 36, 144)  # l-block offsets in the 324-wide EY free dim (36 bases)
LOFF_OUT = (0, 32, 128)  # l-block offsets in the 288 output channels
FAN = float(np.sqrt(32.0))
NVOX = 125.0


def _tap_of(b, r):
    """Flat tap index covered by weight-base b at partition group r, or None."""
    if b >= 35:
        return None  # pad base (even-N requirement for fp32r matmul)
    if b < 25:
        dx, dy = divmod(b, 5)
        if (dx, dy, r) == (2, 2, 2):
            return None  # center tap: replaced by w_lin
        return (dx * 5 + dy) * 5 + r  # dz = r
    dx, yb = divmod(b - 25, 2)
    if yb == 1 and r < 3:
        return None  # duplicate of the yb=0 bar
    dy = yb + r
    return (dx * 5 + dy) * 5 + 4  # dz = 4


def _host_consts():
    """EY[k, r, col]: tap-selected emb x spherical-harmonic products (f32)."""
    c = np.arange(-2.0, 3.0)
    lat = np.stack(np.meshgrid(c, c, c, indexing="ij"), axis=-1).reshape(125, 3)
    rad = np.linalg.norm(lat, axis=-1)
    u = lat / np.where(rad == 0.0, 1.0, rad)[:, None]
    ux, uy, uz = u[:, 0], u[:, 1], u[:, 2]

    y0 = np.ones((125, 1))
    y1 = np.sqrt(3.0) * np.stack([uy, uz, ux], axis=-1)
    y2 = np.stack(
        [
            np.sqrt(15.0) * ux * uy,
            np.sqrt(15.0) * uy * uz,
            (np.sqrt(5.0) / 2.0) * (2.0 * uz**2 - ux**2 - uy**2),
            np.sqrt(15.0) * ux * uz,
            (np.sqrt(15.0) / 2.0) * (ux**2 - uy**2),
        ],
        axis=-1,
    )
    ys = (y0, y1, y2)

    # e3nn soft_one_hot_linspace, basis='smooth_finite'
    values = np.linspace(0.0, 2.5, NRB + 2)
    step = values[1] - values[0]
    values = values[1:-1]
    d = (rad[:, None] - values[None, :]) / step

    def sus(x):
        return np.where(x > 0.0, np.exp(-1.0 / np.where(x > 0.0, x, 1.0)), 0.0)

    emb = 1.14136 * np.exp(2.0) * sus(d + 1.0) * sus(1.0 - d)  # [125, 8]
    emb = emb / (NVOX * FAN)

    # EY4[r*8+k, col]: block-diag-matched moving operand, per l-block of 36
    # bases (b=35 is a zero pad column group).
    ey = np.zeros((32, 324), np.float32)
    for r in range(4):
        for l in range(3):
            ml = MLS[l]
            for b in range(36):
                t = _tap_of(b, r)
                if t is None:
                    continue
                col = LOFF_EY[l] + b * ml
                ey[r * 8 : r * 8 + 8, col : col + ml] = (
                    emb[t, :, None] * ys[l][t, None, :]
                ).astype(np.float32)
    return ey


def _build_nc(repeat=1):
    nc = bacc.Bacc("TRN2", target_bir_lowering=False, debug=False)

    pa_d = nc.dram_tensor("pa", [128, XS, 36, 32], F16, kind="ExternalInput")
    pb_d = nc.dram_tensor("pb", [128, XS, 33, 32], F16, kind="ExternalInput")
    w_d = [
        nc.dram_tensor(f"w{l}t", [NRB, 32, 32], F32R, kind="ExternalInput")
        for l in range(3)
    ]
    wlin_d = nc.dram_tensor("wlin", [32, 32], F32, kind="ExternalInput")
    ey_d = nc.dram_tensor("ey", [32, 324], F32R, kind="ExternalInput")
    out_d = nc.dram_tensor("out", [XPER * 32 * 32, COUT], F32, kind="ExternalOutput")

    with tile.TileContext(nc) as tc:
        with (
            tc.tile_pool(name="wpool", bufs=1) as wpool,
            tc.tile_pool(name="rall", bufs=1) as rall_pool,
            tc.tile_pool(name="slab", bufs=1) as slab_pool,
            tc.tile_pool(name="stage", bufs=3) as stage_pool,
            tc.tile_pool(name="ps", bufs=6, space="PSUM") as ps_pool,
        ):
            # --- tiny inputs ---
            ey_sb = wpool.tile([32, 324], F32R, tag="ey", name="ey_sb")
            nc.sync.dma_start(ey_sb[:], ey_d[:])
            wlin_sb = wpool.tile([128, 32], F32, tag="wlin", name="wlin_sb")
            nc.sync.dma_start(wlin_sb[64:96, :], wlin_d[:])

            # Block-diagonal stationary: BD_l[8r+k, v*128 + 32r+u] = w_l[k,u,v]
            bd_sb = []
            for l in range(3):
                bd_l = wpool.tile([32, 32 * 128], F32R, tag=f"bd{l}", name=f"bd{l}")
                nc.vector.memset(bd_l[:].bitcast(F32), 0.0)
                bd3 = bd_l.rearrange("p (v q) -> p v q", q=128)
                for r in range(4):
                    nc.sync.dma_start(
                        bd3[8 * r : 8 * r + 8, :, 32 * r : 32 * r + 32], w_d[l][:]
                    )
                bd_sb.append(bd_l)

            # --- conv-weight synthesis into R_all[(r,u), (b, outch)] ---
            # one fp32r matmul per (l, v): [K=32 BD] x [K=32, N=36*ml EY] ->
            # psum[(r,u), (b, m)], then a strided DVE copy into R_all.
            r_all = rall_pool.tile([128, 35 * COUT], F16, tag="rall", name="r_all")
            r_view = r_all.rearrange("p (b c) -> p b c", c=COUT)
            for l in range(3):
                ml = MLS[l]
                for v in range(32):
                    ps_lv = ps_pool.tile([128, 36 * ml], F32, tag="ps", name="ps_syn")
                    nc.tensor.matmul(
                        ps_lv[:, :],
                        bd_sb[l][:, 128 * v : 128 * (v + 1)],
                        ey_sb[:, LOFF_EY[l] : LOFF_EY[l] + 36 * ml],
                        start=True,
                        stop=True,
                    )
                    nc.vector.tensor_copy(
                        r_view[:, :, LOFF_OUT[l] + v * ml : LOFF_OUT[l] + (v + 1) * ml],
                        ps_lv.rearrange("p (b m) -> p b m", m=ml)[:, :35, :],
                    )
            # center tap (2,2,2) -> base b=12, group r=2, l=0 block
            nc.scalar.mul(
                r_view[64:96, 12, 0:32],
                wlin_sb[64:96, :],
                1.0 / FAN,
            )

            # --- input slabs (4x partition-replicated with shifts) ---
            a_sb = []
            b_sb = []
            for x in range(XS):
                a_x = slab_pool.tile([128, 36, 32], F16, tag=f"A{x}", name=f"a{x}")
                nc.sync.dma_start(a_x[:], pa_d[:, x, :, :])
                a_sb.append(a_x)
                b_x = slab_pool.tile([128, 33, 32], F16, tag=f"B{x}", name=f"b{x}")
                nc.sync.dma_start(b_x[:], pb_d[:, x, :, :])
                b_sb.append(b_x)

            # --- conv main loop: 64 blocks x 35 accumulating matmuls ---
            # (repeat>1 re-runs the whole conv pass; benchmarking only)
            for _rep in range(repeat):
              for xo in range(XPER):
                for yb in range(8):
                    ps_blk = ps_pool.tile([128, COUT], F32, tag="ps", name="ps_blk")
                    mm = 0
                    for dx in range(5):
                        for dy in range(5):
                            b = dx * 5 + dy
                            y0 = 4 * yb + dy
                            nc.tensor.matmul(
                                ps_blk[:, :],
                                a_sb[xo + dx][:, y0 : y0 + 4, :],
                                r_all[:, b * COUT : (b + 1) * COUT],
                                start=(mm == 0),
                                stop=False,
                            )
                            mm += 1
                    for dx in range(5):
                        for yb2 in range(2):
                            b = 25 + dx * 2 + yb2
                            y0 = 4 * yb + yb2
                            mm += 1
                            nc.tensor.matmul(
                                ps_blk[:, :],
                                b_sb[xo + dx][:, y0 : y0 + 4, :],
                                r_all[:, b * COUT : (b + 1) * COUT],
                                start=False,
                                stop=(mm == 35),
                            )
                    stg = stage_pool.tile([128, COUT], F32, tag="stg", name="stg")
                    nc.vector.tensor_copy(stg[:], ps_blk[:])
                    row = xo * 1024 + yb * 128
                    nc.sync.dma_start(out_d[row : row + 128, :], stg[:])

    nc.compile()
    return nc


def _shard_inputs(x, w0, w1, w2, w_lin):
    ey = _host_consts()
    wts = [
        np.ascontiguousarray(w.transpose(0, 2, 1)).astype(np.float32)
        for w in (w0, w1, w2)
    ]
    w_lin = np.ascontiguousarray(w_lin).astype(np.float32)
    in_maps = []
    for core in range(8):
        bb, xi = divmod(core, 4)
        x0 = xi * XPER
        pp = np.zeros((CIN, XS, 36, 36), np.float32)
        glo, ghi = x0 - 2, x0 + XPER + 2
        slo, shi = max(glo, 0), min(ghi, GRID)
        pp[:, slo - glo : shi - glo, 2:34, 2:34] = x[bb, slo:shi].transpose(3, 0, 1, 2)
        p4a = np.stack([pp[:, :, :, r : r + 32] for r in range(4)], axis=0)
        p4b = np.stack([pp[:, :, r : r + 33, 4:36] for r in range(4)], axis=0)
        in_maps.append(
            {
                "pa": np.ascontiguousarray(p4a).reshape(128, XS, 36, 32).astype(np.float16),
                "pb": np.ascontiguousarray(p4b).reshape(128, XS, 33, 32).astype(np.float16),
                "w0t": wts[0],
                "w1t": wts[1],
                "w2t": wts[2],
                "wlin": w_lin,
                "ey": ey,
            }
        )
    return in_maps


_NC = None


def _run(x, w0, w1, w2, w_lin, **spmd_kwargs):
    global _NC
    if _NC is None:
        _NC = _build_nc()
    in_maps = _shard_inputs(
        np.asarray(x, np.float32),
        np.asarray(w0, np.float32),
        np.asarray(w1, np.float32),
        np.asarray(w2, np.float32),
        np.asarray(w_lin, np.float32),
    )
    res = run_bass_kernel_spmd(_NC, in_maps, core_ids=list(range(8)), **spmd_kwargs)
    out = np.empty((2, GRID, GRID, GRID, COUT), np.float32)
    for core in range(8):
        bb, xi = divmod(core, 4)
        out[bb, xi * XPER : (xi + 1) * XPER] = res.results[core]["out"].reshape(
            XPER, GRID, GRID, COUT
        )
    return out, res


def kernel(x, w0, w1, w2, w_lin):
    out, _ = _run(x, w0, w1, w2, w_lin)
    return out
